# revision 2
# baseline (speedup 1.0000x reference)
"""Trainium2 Bass kernel for nn_MultiHeadAttention (B=2, T=2048, M=2048, H=16, D=128).

Sharding: 8 cores = batch(2) x head-groups(4); host sums the 4 head-group
partials per batch (bf16 partials, fp32 sum).

Design (v2): transpose-free dataflow.
- Host supplies xT (=x.T) in bf16 plus bf16 weights; rope tables in [d, t]
  layout (cos rows 0-63, sin rows 64-127).
- Phase 1 computes qT/kT [d, t] DIRECTLY (lhsT = wq chunk, rhs = xT chunk);
  RMS-norm mean-of-squares via a ones[128,128] matmul that lands the
  partition-sum broadcast across all partitions; norm scale is folded into
  the rope multiplies.  v is computed in [t, (h d)] layout (lhsT = xT tile).
- Phase 2 computes S transposed (sT[j, i] = kT.T @ qT), so softmaxed
  exp(sT) feeds the PV matmul directly as the moving operand (no p
  transposes).  Softmax denominators via the same ones-matmul trick
  (broadcast down all partitions); normalization applied to oT at PSUM
  evacuation.  i-blocks processed in pairs (256 free) over all 4 heads.
- Phase 3 is the standard out-projection from oT [d, h, t].

All matmuls bf16 (fp32 PSUM accum); norm/rope/softmax fp32 on ACT/DVE.
"""
import sys

BASS_PATH = "/opt/trn_rl_repo"
if BASS_PATH not in sys.path:
    sys.path.insert(0, BASS_PATH)

import numpy as np
from contextlib import ExitStack

import concourse.bass as bass
import concourse.tile as tile
from concourse import mybir
from concourse.bass_utils import run_bass_kernel_spmd
from concourse.vector_clock import ScopedClock

FP32 = mybir.dt.float32
BF16 = mybir.dt.bfloat16
FP8 = mybir.dt.float8e4
DR = mybir.MatmulPerfMode.DoubleRow

B, T, M, D = 2, 2048, 2048, 128
H = M // D                      # 16 heads total
HPC = 4                         # heads per core
JW = HPC * D                    # 512
N_CORES = 8
QK_SCALE = 1.0 / D
NORM_EPS = 1e-6
ROTARY_BASE = 10000.0
NEG_INF = -1e30
NM = M // 128                   # 16 contraction chunks


def _max_waits(inst):
    return 1


class SplitDrainTileContext(tile.TileContext):
    """TileContext that splits excess sem waits across nofuse NOPs."""

    def _commit_and_lower(self, inst, original_block, old_bb_map, bb_to_exit_bb):
        si = getattr(inst, "sync_info", None)
        eng = getattr(inst, "engine", None)
        cap = _max_waits(inst)
        if (si is not None and si.on_wait and len(si.on_wait) > cap
                and eng is not None and eng != mybir.EngineType.Unassigned):
            waits = list(si.on_wait)
            excess, keep = waits[:-cap], waits[-cap:]
            inst.sync_info = mybir.SyncInfo(
                on_wait=keep, on_update=list(si.on_update or []))
            for w in excess:
                nop = mybir.InstNoOp(
                    name=self.nc.get_next_instruction_name(),
                    engine=eng,
                    bass_nofuse=True,
                    sync_info=mybir.SyncInfo(on_wait=[w], on_update=[]),
                )
                super()._commit_and_lower(nop, original_block, old_bb_map,
                                          bb_to_exit_bb)
        return super()._commit_and_lower(inst, original_block, old_bb_map,
                                         bb_to_exit_bb)

    def _drain_and_barrier(self, tick_clock, wait_clock):
        probe = self.nc.sync.nop(nofuse=True)
        wait_clock.add_sem_waits(probe.ins, ScopedClock({None: tick_clock.global_clock}))
        si = probe.ins.sync_info
        waits = list(si.on_wait) if si and si.on_wait else []
        updates = list(si.on_update) if si and si.on_update else []
        if len(waits) > 1:
            probe.ins.sync_info = mybir.SyncInfo(on_wait=waits[:1], on_update=updates)
            for w in waits[1:]:
                nop = self.nc.sync.nop(nofuse=True)
                nop.ins.sync_info = mybir.SyncInfo(on_wait=[w], on_update=[])
        self.nc.sync.drain()
        self.nc.all_engine_barrier()
        popped = self.nc._tile_sem_poison_stack.pop()
        assert popped is self._sem_poison
        self.nc.clear_and_free_semaphores(list(self.sems.allocated().values()))
        self.nc.all_engine_barrier()


def build_nc(t_len=T, reps=1):
    NT = t_len // 128           # t-blocks
    NG2 = t_len // 512          # 512-wide projection groups
    NP = NT // 2                # attention i-pairs

    nc = bass.Bass()
    xT_d = nc.declare_dram_parameter("xT", [M, t_len], BF16, isOutput=False)
    xq8_d = nc.declare_dram_parameter("xq8", [M, t_len], FP8, isOutput=False)
    wq_d = nc.declare_dram_parameter("wq", [M, JW], FP8, isOutput=False)
    wk_d = nc.declare_dram_parameter("wk", [M, JW], FP8, isOutput=False)
    wv_d = nc.declare_dram_parameter("wv", [M, JW], BF16, isOutput=False)
    wo_d = nc.declare_dram_parameter("wo", [JW, M], BF16, isOutput=False)
    # rope tables: cs_cs = [cos; sin] stacked on partitions, cs_sc = [sin; cos]
    cs_cs_d = nc.declare_dram_parameter("cs_cs", [128, t_len], BF16, isOutput=False)
    cs_sc_d = nc.declare_dram_parameter("cs_sc", [128, t_len], BF16, isOutput=False)
    out_d = nc.declare_dram_parameter("out", [t_len, M], BF16, isOutput=True)

    with SplitDrainTileContext(nc) as tc, ExitStack() as top:
        const_pool = top.enter_context(tc.tile_pool(name="const", bufs=1))
        ones_sb = const_pool.tile([128, 128], BF16, tag="ones")
        nc.vector.memset(ones_sb, 1.0)
        # sT[j, i] mask: fill -inf where j > i (keep where f - p >= 0)
        negmaskT = const_pool.tile([128, 1, 128], FP32, tag="negmaskT")
        nc.gpsimd.memset(negmaskT, 0.0)
        nc.gpsimd.affine_select(
            out=negmaskT, in_=negmaskT,
            compare_op=mybir.AluOpType.is_ge,
            fill=NEG_INF, base=0,
            pattern=[[0, 1], [1, 128]], channel_multiplier=-1,
        )
        eps_t = const_pool.tile([128, 1], FP32, tag="eps")
        nc.vector.memset(eps_t, NORM_EPS)
        cs_cs = const_pool.tile([128, 1, t_len], BF16, tag="cs_cs")
        cs_sc = const_pool.tile([128, 1, t_len], BF16, tag="cs_sc")
        nc.sync.dma_start(out=cs_cs, in_=cs_cs_d.rearrange("p (a t) -> p a t", a=1))
        nc.sync.dma_start(out=cs_sc, in_=cs_sc_d.rearrange("p (a t) -> p a t", a=1))

        act_pool = top.enter_context(tc.tile_pool(name="acts", bufs=1))
        # qkT[d, qk, h, t]
        qkT = act_pool.tile([128, 2, HPC, t_len], BF16, tag="qkT")
        v_sb = act_pool.tile([128, NT, JW], BF16, tag="v")
        oT = act_pool.tile([128, HPC, t_len], BF16, tag="oT")

        def _phase1():
          with ExitStack() as outer:
            wvpool = outer.enter_context(tc.tile_pool(name="wv", bufs=1))
            wv_sb = wvpool.tile([128, NM, JW], BF16, tag="wv")
            xvpool = outer.enter_context(tc.tile_pool(name="xv", bufs=3))
            vps = outer.enter_context(
                tc.tile_pool(name="v_ps", bufs=2, space=bass.MemorySpace.PSUM))
            xvs = {}

            def emit_xv(ti):
                xv = xvpool.tile([128, NM, 128], BF16, tag="xv")
                nc.sync.dma_start(
                    out=xv,
                    in_=xT_d[:, ti * 128:(ti + 1) * 128].rearrange(
                        "(c p) t -> p c t", p=128))
                xvs[ti] = xv

            with ExitStack() as st:
                wpool = st.enter_context(tc.tile_pool(name="wqk", bufs=1))
                wq_sb = wpool.tile([128, NM // 2, 2, JW], FP8, tag="wq")
                wk_sb = wpool.tile([128, NM // 2, 2, JW], FP8, tag="wk")
                xpool = st.enter_context(tc.tile_pool(name="xTs", bufs=2))
                ppool = st.enter_context(
                    tc.tile_pool(name="qk_ps", bufs=1, space=bass.MemorySpace.PSUM))
                mpool = st.enter_context(
                    tc.tile_pool(name="msq_ps", bufs=1, space=bass.MemorySpace.PSUM))
                work = st.enter_context(tc.tile_pool(name="p1w", bufs=2))

                for mp in range(NM // 2):
                    nc.sync.dma_start(
                        out=wq_sb[:, mp, :, :],
                        in_=wq_d[mp * 256:(mp + 1) * 256, :].rearrange(
                            "(j p) c -> p j c", p=128))
                    nc.sync.dma_start(
                        out=wk_sb[:, mp, :, :],
                        in_=wk_d[mp * 256:(mp + 1) * 256, :].rearrange(
                            "(j p) c -> p j c", p=128))
                for m in range(NM):
                    nc.sync.dma_start(out=wv_sb[:, m, :],
                                      in_=wv_d[m * 128:(m + 1) * 128, :])

                # 2-stage software pipeline over (g2, h) iterations:
                # burst(i) | msq(i-1) | combine(i-2) on the PE; the
                # norm/rope chain runs off-PE in the gaps.
                iters = [(g2, h) for g2 in range(NG2) for h in range(HPC)]
                xts = {}
                stt = {}

                def emit_burst(i):
                    g2, h = iters[i]
                    lo = g2 * 512
                    if h == 0:
                        xt = xpool.tile([128, NM // 2, 2, 512], FP8, tag="xT")
                        for mp in range(NM // 2):
                            nc.sync.dma_start(
                                out=xt[:, mp, :, :],
                                in_=xq8_d[mp * 256:(mp + 1) * 256,
                                          lo:lo + 512].rearrange(
                                    "(j p) t -> p j t", p=128))
                        xts[g2] = xt
                    xt = xts[g2]
                    ps = ppool.tile([128, 2, 512], FP32, name="ps",
                                    tag=f"ps{i % 2}", bufs=1)
                    for mp in range(NM // 2):
                        nc.tensor.matmul(ps[:, 0, :],
                                         wq_sb[:, mp, :, h * D:(h + 1) * D],
                                         xt[:, mp, :, :], start=(mp == 0),
                                         stop=(mp == NM // 2 - 1), perf_mode=DR)
                        nc.tensor.matmul(ps[:, 1, :],
                                         wk_sb[:, mp, :, h * D:(h + 1) * D],
                                         xt[:, mp, :, :], start=(mp == 0),
                                         stop=(mp == NM // 2 - 1), perf_mode=DR)
                    # evacuate psum right away (frees the bank pair) + square
                    psf = work.tile([128, 2, 512], FP32, tag="psf")
                    nc.scalar.copy(out=psf, in_=ps)
                    sq = work.tile([128, 2, 512], BF16, tag="sq")
                    nc.scalar.activation(out=sq, in_=psf,
                                         func=mybir.ActivationFunctionType.Square)
                    stt[i] = {"lo": lo, "h": h, "psf": psf, "sq": sq}

                def emit_post(i):
                    s = stt.pop(i)
                    lo, h = s["lo"], s["h"]
                    msq = mpool.tile([128, 2, 512], FP32, tag="msq")
                    nc.tensor.matmul(msq[:, 0, :], ones_sb, s["sq"][:, 0, :])
                    nc.tensor.matmul(msq[:, 1, :], ones_sb, s["sq"][:, 1, :])
                    # rstd = exp(-0.5 * ln(msq/D + eps)) — ln and exp share
                    # one ACT table set (no usable rsqrt table here)
                    lnm = work.tile([128, 2, 512], FP32, tag="lnm", bufs=1)
                    nc.scalar.activation(
                        out=lnm, in_=msq,
                        func=mybir.ActivationFunctionType.Ln,
                        scale=1.0 / D, bias=eps_t)
                    rstd = work.tile([128, 2, 512], FP32, tag="rstd", bufs=1)
                    nc.scalar.activation(
                        out=rstd, in_=lnm,
                        func=mybir.ActivationFunctionType.Exp, scale=-0.5)
                    qn = work.tile([128, 2, 512], BF16, tag="qn", bufs=1)
                    nc.vector.tensor_mul(qn, s["psf"], rstd)
                    # rope via cross-partition copies (TensorTensor ops must
                    # share a start partition; copies may cross):
                    #   oc = odd half at base 0, ec = even half at base 64
                    oc = work.tile([128, 2, 512], BF16, tag="oc", bufs=1)
                    ec = work.tile([128, 2, 512], BF16, tag="ec", bufs=1)
                    nc.vector.tensor_copy(out=oc[0:64], in_=qn[64:128])
                    nc.scalar.copy(out=ec[64:128], in_=qn[0:64])
                    cos0 = cs_cs[0:64, :, lo:lo + 512].to_broadcast([64, 2, 512])
                    sin0 = cs_sc[0:64, :, lo:lo + 512].to_broadcast([64, 2, 512])
                    sin64 = cs_cs[64:128, :, lo:lo + 512].to_broadcast([64, 2, 512])
                    cos64 = cs_sc[64:128, :, lo:lo + 512].to_broadcast([64, 2, 512])
                    t1 = work.tile([128, 2, 512], BF16, tag="t1", bufs=1)
                    t2 = work.tile([128, 2, 512], BF16, tag="t2", bufs=1)
                    t3 = work.tile([128, 2, 512], BF16, tag="t3", bufs=1)
                    t4 = work.tile([128, 2, 512], BF16, tag="t4", bufs=1)
                    nc.vector.tensor_mul(t1[0:64], qn[0:64], cos0)
                    nc.vector.tensor_mul(t2[0:64], oc[0:64], sin0)
                    nc.vector.tensor_mul(t3[64:128], ec[64:128], sin64)
                    nc.vector.tensor_mul(t4[64:128], qn[64:128], cos64)
                    nc.vector.tensor_sub(qkT[0:64, :, h, lo:lo + 512],
                                          t1[0:64], t2[0:64])
                    nc.vector.tensor_add(qkT[64:128, :, h, lo:lo + 512],
                                         t3[64:128], t4[64:128])

                n = len(iters)
                for i in range(n + 1):
                    if i < n:
                        emit_burst(i)
                    if i == n - 1:
                        emit_xv(0)
                        emit_xv(1)
                    if 0 <= i - 1 < n:
                        emit_post(i - 1)

            # ---- v projection: v[t, (h d)] ----
            for ti in range(NT):
                if ti + 2 < NT:
                    emit_xv(ti + 2)
                ps_v = vps.tile([128, JW], FP32, tag="psv")
                for m in range(NM):
                    nc.tensor.matmul(ps_v, xvs[ti][:, m, :], wv_sb[:, m, :],
                                     start=(m == 0), stop=(m == NM - 1))
                del xvs[ti]
                nc.scalar.copy(out=v_sb[:, ti, :], in_=ps_v)

        def _phase23():
            with ExitStack() as st:
                wopool = st.enter_context(tc.tile_pool(name="wo", bufs=1))
                wo_sb = wopool.tile([128, HPC, M], BF16, tag="wo")
                for h in range(HPC):
                    nc.sync.dma_start(out=wo_sb[:, h, :],
                                      in_=wo_d[h * D:(h + 1) * D, :])

                with ExitStack() as ph2:
                    spool = ph2.enter_context(
                        tc.tile_pool(name="s_ps", bufs=2, space=bass.MemorySpace.PSUM))
                    opool = ph2.enter_context(
                        tc.tile_pool(name="o_ps", bufs=1, space=bass.MemorySpace.PSUM))
                    dpool = ph2.enter_context(
                        tc.tile_pool(name="d_ps", bufs=1, space=bass.MemorySpace.PSUM))
                    epool = ph2.enter_context(tc.tile_pool(name="expT", bufs=3))
                    rpool = ph2.enter_context(tc.tile_pool(name="recip", bufs=2))

                    steps = [(P, jb) for P in range(NP) for jb in range(2 * P + 2)]
                    ot = {}
                    dt = {}

                    def emit_s(P, jb):
                        """S matmuls (+ causal mask) for step (P, jb)."""
                        s_t = spool.tile([128, HPC, 256], FP32, tag="s")
                        diag0, diag1 = (jb == 2 * P), (jb == 2 * P + 1)
                        for h in range(HPC):
                            if diag1:
                                nc.tensor.matmul(
                                    s_t[:, h, 128:256],
                                    qkT[:, 1, h, jb * 128:(jb + 1) * 128],
                                    qkT[:, 0, h, P * 256 + 128:P * 256 + 256])
                            else:
                                nc.tensor.matmul(
                                    s_t[:, h, 0:256],
                                    qkT[:, 1, h, jb * 128:(jb + 1) * 128],
                                    qkT[:, 0, h, P * 256:(P + 1) * 256])
                        if diag0:
                            nc.vector.tensor_add(
                                s_t[:, :, 0:128], s_t[:, :, 0:128],
                                negmaskT.to_broadcast([128, HPC, 128]))
                        if diag1:
                            nc.vector.tensor_add(
                                s_t[:, :, 128:256], s_t[:, :, 128:256],
                                negmaskT.to_broadcast([128, HPC, 128]))
                        return s_t

                    s_cur = emit_s(*steps[0])
                    for si, (P, jb) in enumerate(steps):
                        if jb == 0:
                            ot[P] = opool.tile([128, HPC, 256], FP32, name="o_t", tag="o")
                            dt[P] = dpool.tile([128, HPC, 256], FP32, name="d_t", tag="d")
                        diag1 = (jb == 2 * P + 1)
                        s_next = (emit_s(*steps[si + 1])
                                  if si + 1 < len(steps) else None)
                        # exp split into h-pairs so den/PV of the first pair
                        # can start while the second half is still exp'ing
                        e_t = epool.tile([128, HPC, 256], BF16, tag="e")
                        cl, ch = (128, 256) if diag1 else (0, 256)
                        last = diag1
                        for hp in range(2):
                            h0 = 2 * hp
                            nc.scalar.activation(
                                out=e_t[:, h0:h0 + 2, cl:ch],
                                in_=s_cur[:, h0:h0 + 2, cl:ch],
                                func=mybir.ActivationFunctionType.Exp,
                                scale=QK_SCALE)
                            for h in (h0, h0 + 1):
                                nc.tensor.matmul(
                                    dt[P][:, h, cl:ch], ones_sb, e_t[:, h, cl:ch],
                                    start=(jb == 0 and h % 2 == 0), stop=last,
                                    skip_group_check=True)
                            for h in (h0, h0 + 1):
                                nc.tensor.matmul(
                                    ot[P][:, h, cl:ch],
                                    v_sb[:, jb, h * D:(h + 1) * D],
                                    e_t[:, h, cl:ch],
                                    start=(jb == 0 and h % 2 == 0), stop=last,
                                    skip_group_check=True)
                        if last:
                            # recip = exp(-ln(den)); ln is the only dt reader
                            # so the den banks free early
                            lnd = rpool.tile([128, HPC, 256], FP32, tag="lnd")
                            nc.scalar.activation(
                                out=lnd, in_=dt[P],
                                func=mybir.ActivationFunctionType.Ln)
                            rec0 = rpool.tile([128, HPC, 256], FP32, tag="rec0")
                            nc.scalar.activation(
                                out=rec0, in_=lnd,
                                func=mybir.ActivationFunctionType.Exp, scale=-1.0)
                            for hp in range(2):
                                h0 = 2 * hp
                                nc.vector.tensor_mul(
                                    oT[:, h0:h0 + 2, P * 256:(P + 1) * 256],
                                    ot[P][:, h0:h0 + 2, :], rec0[:, h0:h0 + 2, :])
                            del ot[P], dt[P]
                        s_cur = s_next

                # ---- phase 3: out = sum_h oT_h.T @ wo_h ----
                with ExitStack() as ph3:
                    upool = ph3.enter_context(
                        tc.tile_pool(name="u_ps", bufs=2, space=bass.MemorySpace.PSUM))
                    ostage = ph3.enter_context(tc.tile_pool(name="ostg", bufs=2))
                    for ti in range(NT):
                        ps_u = upool.tile([128, M], FP32, tag="u")
                        for mc in range(M // 512):
                            for h in range(HPC):
                                nc.tensor.matmul(
                                    ps_u[:, mc * 512:(mc + 1) * 512],
                                    oT[:, h, ti * 128:(ti + 1) * 128],
                                    wo_sb[:, h, mc * 512:(mc + 1) * 512],
                                    start=(h == 0), stop=(h == HPC - 1))
                        o_sb = ostage.tile([128, M], BF16, tag="osb")
                        nc.scalar.copy(out=o_sb[:, 0:1024], in_=ps_u[:, 0:1024])
                        nc.vector.tensor_copy(out=o_sb[:, 1024:2048],
                                              in_=ps_u[:, 1024:2048])
                        nc.sync.dma_start(out=out_d[ti * 128:(ti + 1) * 128, :],
                                          in_=o_sb)

        for _ in range(reps):
            _phase1()
            _phase23()

    return nc


def rope_consts(t_len=T):
    """cs_cs=[cos;sin], cs_sc=[sin;cos] ([128, t_len] bf16), c1=[I;-I], c2=[I;I]."""
    import ml_dtypes
    bf16 = ml_dtypes.bfloat16
    pos = np.arange(t_len, dtype=np.float64)[None, :]
    dims = np.arange(D // 2, dtype=np.float64)[:, None]
    freqs = ROTARY_BASE ** (-dims / (D // 2))
    rad = freqs * pos                              # [64, t_len]
    c, s = np.cos(rad), np.sin(rad)
    cs_cs = np.ascontiguousarray(np.concatenate([c, s]).astype(bf16))
    cs_sc = np.ascontiguousarray(np.concatenate([s, c]).astype(bf16))
    eye = np.eye(64, dtype=np.float32)
    c1 = np.ascontiguousarray(np.concatenate([eye, -eye]).astype(bf16))
    c2 = np.ascontiguousarray(np.concatenate([eye, eye]).astype(bf16))
    return cs_cs, cs_sc, c1, c2


_NC_CACHE = {}


def make_in_maps(x, wq, wk, wv, wo, t_len=T):
    import ml_dtypes
    bf16 = ml_dtypes.bfloat16
    fp8 = ml_dtypes.float8_e4m3
    cs_cs, cs_sc, c1, c2 = rope_consts(t_len)
    xTs = [np.ascontiguousarray(np.asarray(x[b]).T.astype(bf16)) for b in range(B)]
    # q/k projections are RMS-normed afterwards, so fp8 scaling cancels:
    # scale weights by 256 to clear the e4m3 subnormal range
    xq8s = [np.ascontiguousarray(np.asarray(x[b]).T.astype(fp8)) for b in range(B)]
    in_maps = []
    for c in range(N_CORES):
        b, g = divmod(c, N_CORES // B)
        hs = slice(g * HPC, (g + 1) * HPC)
        in_maps.append({
            "xT": xTs[b],
            "xq8": xq8s[b],
            "wq": np.ascontiguousarray(
                (wq[:, hs, :].reshape(M, JW) * 256.0).astype(fp8)),
            "wk": np.ascontiguousarray(
                (wk[:, hs, :].reshape(M, JW) * 256.0).astype(fp8)),
            "wv": np.ascontiguousarray(wv[:, hs, :].reshape(M, JW).astype(bf16)),
            "wo": np.ascontiguousarray(wo[hs].reshape(JW, M).astype(bf16)),
            "cs_cs": cs_cs, "cs_sc": cs_sc, "c1": c1, "c2": c2,
        })
    return in_maps


def kernel(x, wq, wk, wv, wo):
    if T not in _NC_CACHE:
        _NC_CACHE[T] = build_nc(T)
    nc = _NC_CACHE[T]
    in_maps = make_in_maps(x, wq, wk, wv, wo)
    res = run_bass_kernel_spmd(nc, in_maps, list(range(N_CORES)))
    gpb = N_CORES // B
    out = np.stack([
        sum(res.results[b * gpb + g]["out"].astype(np.float64) for g in range(gpb))
        for b in range(B)
    ]).astype(np.float32)
    return out


# revision 3
# speedup vs baseline: 1.0510x; 1.0510x over previous
"""Trainium2 Bass kernel for nn_MultiHeadAttention (B=2, T=2048, M=2048, H=16, D=128).

Sharding: 8 cores = batch(2) x head-groups(4); host sums the 4 head-group
partials per batch (bf16 partials, fp32 sum).

Design (v2): transpose-free dataflow.
- Host supplies xT (=x.T) in bf16 plus bf16 weights; rope tables in [d, t]
  layout (cos rows 0-63, sin rows 64-127).
- Phase 1 computes qT/kT [d, t] DIRECTLY (lhsT = wq chunk, rhs = xT chunk);
  RMS-norm mean-of-squares via a ones[128,128] matmul that lands the
  partition-sum broadcast across all partitions; norm scale is folded into
  the rope multiplies.  v is computed in [t, (h d)] layout (lhsT = xT tile).
- Phase 2 computes S transposed (sT[j, i] = kT.T @ qT), so softmaxed
  exp(sT) feeds the PV matmul directly as the moving operand (no p
  transposes).  Softmax denominators via the same ones-matmul trick
  (broadcast down all partitions); normalization applied to oT at PSUM
  evacuation.  i-blocks processed in pairs (256 free) over all 4 heads.
- Phase 3 is the standard out-projection from oT [d, h, t].

All matmuls bf16 (fp32 PSUM accum); norm/rope/softmax fp32 on ACT/DVE.
"""
import sys

BASS_PATH = "/opt/trn_rl_repo"
if BASS_PATH not in sys.path:
    sys.path.insert(0, BASS_PATH)

import numpy as np
from contextlib import ExitStack

import concourse.bass as bass
import concourse.tile as tile
from concourse import mybir
from concourse.bass_utils import run_bass_kernel_spmd
from concourse.vector_clock import ScopedClock

FP32 = mybir.dt.float32
BF16 = mybir.dt.bfloat16
FP8 = mybir.dt.float8e4
DR = mybir.MatmulPerfMode.DoubleRow

B, T, M, D = 2, 2048, 2048, 128
H = M // D                      # 16 heads total
HPC = 4                         # heads per core
JW = HPC * D                    # 512
N_CORES = 8
QK_SCALE = 1.0 / D
NORM_EPS = 1e-6
ROTARY_BASE = 10000.0
NEG_INF = -1e30
NM = M // 128                   # 16 contraction chunks


def _max_waits(inst):
    return 1


class SplitDrainTileContext(tile.TileContext):
    """TileContext that splits excess sem waits across nofuse NOPs."""

    def _commit_and_lower(self, inst, original_block, old_bb_map, bb_to_exit_bb):
        si = getattr(inst, "sync_info", None)
        eng = getattr(inst, "engine", None)
        cap = _max_waits(inst)
        if (si is not None and si.on_wait and len(si.on_wait) > cap
                and eng is not None and eng != mybir.EngineType.Unassigned):
            waits = list(si.on_wait)
            excess, keep = waits[:-cap], waits[-cap:]
            inst.sync_info = mybir.SyncInfo(
                on_wait=keep, on_update=list(si.on_update or []))
            for w in excess:
                nop = mybir.InstNoOp(
                    name=self.nc.get_next_instruction_name(),
                    engine=eng,
                    bass_nofuse=True,
                    sync_info=mybir.SyncInfo(on_wait=[w], on_update=[]),
                )
                super()._commit_and_lower(nop, original_block, old_bb_map,
                                          bb_to_exit_bb)
        return super()._commit_and_lower(inst, original_block, old_bb_map,
                                         bb_to_exit_bb)

    def _drain_and_barrier(self, tick_clock, wait_clock):
        probe = self.nc.sync.nop(nofuse=True)
        wait_clock.add_sem_waits(probe.ins, ScopedClock({None: tick_clock.global_clock}))
        si = probe.ins.sync_info
        waits = list(si.on_wait) if si and si.on_wait else []
        updates = list(si.on_update) if si and si.on_update else []
        if len(waits) > 1:
            probe.ins.sync_info = mybir.SyncInfo(on_wait=waits[:1], on_update=updates)
            for w in waits[1:]:
                nop = self.nc.sync.nop(nofuse=True)
                nop.ins.sync_info = mybir.SyncInfo(on_wait=[w], on_update=[])
        self.nc.sync.drain()
        self.nc.all_engine_barrier()
        popped = self.nc._tile_sem_poison_stack.pop()
        assert popped is self._sem_poison
        self.nc.clear_and_free_semaphores(list(self.sems.allocated().values()))
        self.nc.all_engine_barrier()


def build_nc(t_len=T, reps=1):
    NT = t_len // 128           # t-blocks
    NG2 = t_len // 512          # 512-wide projection groups
    NP = NT // 2                # attention i-pairs

    nc = bass.Bass()
    xT_d = nc.declare_dram_parameter("xT", [M, t_len], BF16, isOutput=False)
    xq8_d = nc.declare_dram_parameter("xq8", [M, t_len], FP8, isOutput=False)
    wq_d = nc.declare_dram_parameter("wq", [M, JW], FP8, isOutput=False)
    wk_d = nc.declare_dram_parameter("wk", [M, JW], FP8, isOutput=False)
    wv_d = nc.declare_dram_parameter("wv", [M, JW], BF16, isOutput=False)
    wo_d = nc.declare_dram_parameter("wo", [JW, M], BF16, isOutput=False)
    # rope tables: cs_cs = [cos; sin] stacked on partitions, cs_sc = [sin; cos]
    cs_cs_d = nc.declare_dram_parameter("cs_cs", [128, t_len], BF16, isOutput=False)
    cs_sc_d = nc.declare_dram_parameter("cs_sc", [128, t_len], BF16, isOutput=False)
    out_d = nc.declare_dram_parameter("out", [t_len, M], BF16, isOutput=True)

    with SplitDrainTileContext(nc) as tc, ExitStack() as top:
        const_pool = top.enter_context(tc.tile_pool(name="const", bufs=1))
        ones_sb = const_pool.tile([128, 128], BF16, tag="ones")
        nc.vector.memset(ones_sb, 1.0)
        # sT[j, i] mask: fill -inf where j > i (keep where f - p >= 0)
        negmaskT = const_pool.tile([128, 1, 128], FP32, tag="negmaskT")
        nc.gpsimd.memset(negmaskT, 0.0)
        nc.gpsimd.affine_select(
            out=negmaskT, in_=negmaskT,
            compare_op=mybir.AluOpType.is_ge,
            fill=NEG_INF, base=0,
            pattern=[[0, 1], [1, 128]], channel_multiplier=-1,
        )
        eps_t = const_pool.tile([128, 1], FP32, tag="eps")
        nc.vector.memset(eps_t, NORM_EPS)
        cs_cs = const_pool.tile([128, 1, t_len], BF16, tag="cs_cs")
        cs_sc = const_pool.tile([128, 1, t_len], BF16, tag="cs_sc")
        nc.sync.dma_start(out=cs_cs, in_=cs_cs_d.rearrange("p (a t) -> p a t", a=1))
        nc.sync.dma_start(out=cs_sc, in_=cs_sc_d.rearrange("p (a t) -> p a t", a=1))

        act_pool = top.enter_context(tc.tile_pool(name="acts", bufs=1))
        # qkT[d, qk, h, t]
        qkT = act_pool.tile([128, 2, HPC, t_len], BF16, tag="qkT")
        v_sb = act_pool.tile([128, NT, JW], BF16, tag="v")
        oT = act_pool.tile([128, HPC, t_len], BF16, tag="oT")

        def _phase1():
          with ExitStack() as outer:
            wvpool = outer.enter_context(tc.tile_pool(name="wv", bufs=1))
            wv_sb = wvpool.tile([128, NM, JW], BF16, tag="wv")
            xvpool = outer.enter_context(tc.tile_pool(name="xv", bufs=3))
            vps = outer.enter_context(
                tc.tile_pool(name="v_ps", bufs=2, space=bass.MemorySpace.PSUM))
            xvs = {}

            def emit_xv(ti):
                xv = xvpool.tile([128, NM, 128], BF16, tag="xv")
                nc.sync.dma_start(
                    out=xv,
                    in_=xT_d[:, ti * 128:(ti + 1) * 128].rearrange(
                        "(c p) t -> p c t", p=128))
                xvs[ti] = xv

            with ExitStack() as st:
                wpool = st.enter_context(tc.tile_pool(name="wqk", bufs=1))
                wq_sb = wpool.tile([128, NM // 2, 2, JW], FP8, tag="wq")
                wk_sb = wpool.tile([128, NM // 2, 2, JW], FP8, tag="wk")
                xpool = st.enter_context(tc.tile_pool(name="xTs", bufs=2))
                ppool = st.enter_context(
                    tc.tile_pool(name="qk_ps", bufs=1, space=bass.MemorySpace.PSUM))
                mpool = st.enter_context(
                    tc.tile_pool(name="msq_ps", bufs=1, space=bass.MemorySpace.PSUM))
                work = st.enter_context(tc.tile_pool(name="p1w", bufs=2))

                for mp in range(NM // 2):
                    nc.sync.dma_start(
                        out=wq_sb[:, mp, :, :],
                        in_=wq_d[mp * 256:(mp + 1) * 256, :].rearrange(
                            "(j p) c -> p j c", p=128))
                    nc.sync.dma_start(
                        out=wk_sb[:, mp, :, :],
                        in_=wk_d[mp * 256:(mp + 1) * 256, :].rearrange(
                            "(j p) c -> p j c", p=128))
                for m in range(NM):
                    nc.sync.dma_start(out=wv_sb[:, m, :],
                                      in_=wv_d[m * 128:(m + 1) * 128, :])

                # 2-stage software pipeline over (g2, h) iterations:
                # burst(i) | msq(i-1) | combine(i-2) on the PE; the
                # norm/rope chain runs off-PE in the gaps.
                iters = [(g2, h) for g2 in range(NG2) for h in range(HPC)]
                xts = {}
                stt = {}

                def emit_burst(i):
                    g2, h = iters[i]
                    lo = g2 * 512
                    if h == 0:
                        xt = xpool.tile([128, NM // 2, 2, 512], FP8, tag="xT")
                        for mp in range(NM // 2):
                            nc.sync.dma_start(
                                out=xt[:, mp, :, :],
                                in_=xq8_d[mp * 256:(mp + 1) * 256,
                                          lo:lo + 512].rearrange(
                                    "(j p) t -> p j t", p=128))
                        xts[g2] = xt
                    xt = xts[g2]
                    ps = ppool.tile([128, 2, 512], FP32, name="ps",
                                    tag=f"ps{i % 2}", bufs=1)
                    for mp in range(NM // 2):
                        nc.tensor.matmul(ps[:, 0, :],
                                         wq_sb[:, mp, :, h * D:(h + 1) * D],
                                         xt[:, mp, :, :], start=(mp == 0),
                                         stop=(mp == NM // 2 - 1), perf_mode=DR)
                        nc.tensor.matmul(ps[:, 1, :],
                                         wk_sb[:, mp, :, h * D:(h + 1) * D],
                                         xt[:, mp, :, :], start=(mp == 0),
                                         stop=(mp == NM // 2 - 1), perf_mode=DR)
                    # evacuate psum right away (frees the bank pair) + square
                    psf = work.tile([128, 2, 512], FP32, tag="psf")
                    nc.scalar.copy(out=psf, in_=ps)
                    sq = work.tile([128, 2, 512], BF16, tag="sq")
                    nc.scalar.activation(out=sq, in_=psf,
                                         func=mybir.ActivationFunctionType.Square)
                    stt[i] = {"lo": lo, "h": h, "psf": psf, "sq": sq}

                def emit_post(i):
                    s = stt.pop(i)
                    lo, h = s["lo"], s["h"]
                    msq = mpool.tile([128, 2, 512], FP32, tag="msq")
                    nc.tensor.matmul(msq[:, 0, :], ones_sb, s["sq"][:, 0, :])
                    nc.tensor.matmul(msq[:, 1, :], ones_sb, s["sq"][:, 1, :])
                    # rstd = exp(-0.5 * ln(msq/D + eps)) — ln and exp share
                    # one ACT table set (no usable rsqrt table here)
                    lnm = work.tile([128, 2, 512], FP32, tag="lnm", bufs=1)
                    nc.scalar.activation(
                        out=lnm, in_=msq,
                        func=mybir.ActivationFunctionType.Ln,
                        scale=1.0 / D, bias=eps_t)
                    rstd = work.tile([128, 2, 512], FP32, tag="rstd", bufs=1)
                    nc.scalar.activation(
                        out=rstd, in_=lnm,
                        func=mybir.ActivationFunctionType.Exp, scale=-0.5)
                    qn = work.tile([128, 2, 512], BF16, tag="qn", bufs=1)
                    nc.vector.tensor_mul(qn, s["psf"], rstd)
                    # rope via cross-partition copies (TensorTensor ops must
                    # share a start partition; copies may cross):
                    #   oc = odd half at base 0, ec = even half at base 64
                    oc = work.tile([128, 2, 512], BF16, tag="oc", bufs=1)
                    ec = work.tile([128, 2, 512], BF16, tag="ec", bufs=1)
                    nc.vector.tensor_copy(out=oc[0:64], in_=qn[64:128])
                    nc.scalar.copy(out=ec[64:128], in_=qn[0:64])
                    cos0 = cs_cs[0:64, :, lo:lo + 512].to_broadcast([64, 2, 512])
                    sin0 = cs_sc[0:64, :, lo:lo + 512].to_broadcast([64, 2, 512])
                    sin64 = cs_cs[64:128, :, lo:lo + 512].to_broadcast([64, 2, 512])
                    cos64 = cs_sc[64:128, :, lo:lo + 512].to_broadcast([64, 2, 512])
                    t1 = work.tile([128, 2, 512], BF16, tag="t1", bufs=1)
                    t2 = work.tile([128, 2, 512], BF16, tag="t2", bufs=1)
                    t3 = work.tile([128, 2, 512], BF16, tag="t3", bufs=1)
                    t4 = work.tile([128, 2, 512], BF16, tag="t4", bufs=1)
                    nc.vector.tensor_mul(t1[0:64], qn[0:64], cos0)
                    nc.vector.tensor_mul(t2[0:64], oc[0:64], sin0)
                    nc.vector.tensor_mul(t3[64:128], ec[64:128], sin64)
                    nc.vector.tensor_mul(t4[64:128], qn[64:128], cos64)
                    nc.vector.tensor_sub(qkT[0:64, :, h, lo:lo + 512],
                                          t1[0:64], t2[0:64])
                    nc.vector.tensor_add(qkT[64:128, :, h, lo:lo + 512],
                                         t3[64:128], t4[64:128])

                n = len(iters)
                for i in range(n + 1):
                    if i < n:
                        emit_burst(i)
                    if i == n - 1:
                        emit_xv(0)
                        emit_xv(1)
                    if 0 <= i - 1 < n:
                        emit_post(i - 1)

            # ---- v projection: v[t, (h d)] ----
            for ti in range(NT):
                if ti + 2 < NT:
                    emit_xv(ti + 2)
                ps_v = vps.tile([128, JW], FP32, tag="psv")
                for m in range(NM):
                    nc.tensor.matmul(ps_v, xvs[ti][:, m, :], wv_sb[:, m, :],
                                     start=(m == 0), stop=(m == NM - 1))
                del xvs[ti]
                nc.scalar.copy(out=v_sb[:, ti, :], in_=ps_v)

        def _phase23():
            with ExitStack() as st:
                wopool = st.enter_context(tc.tile_pool(name="wo", bufs=1))
                wo_sb = wopool.tile([128, HPC, M], BF16, tag="wo")
                for h in range(HPC):
                    nc.sync.dma_start(out=wo_sb[:, h, :],
                                      in_=wo_d[h * D:(h + 1) * D, :])

                with ExitStack() as ph2:
                    spool = ph2.enter_context(
                        tc.tile_pool(name="s_ps", bufs=2, space=bass.MemorySpace.PSUM))
                    opool = ph2.enter_context(
                        tc.tile_pool(name="o_ps", bufs=1, space=bass.MemorySpace.PSUM))
                    dpool = ph2.enter_context(
                        tc.tile_pool(name="d_ps", bufs=1, space=bass.MemorySpace.PSUM))
                    epool = ph2.enter_context(tc.tile_pool(name="expT", bufs=3))
                    rpool = ph2.enter_context(tc.tile_pool(name="recip", bufs=2))

                    steps = [(P, jb) for P in range(NP) for jb in range(2 * P + 2)]
                    ot = {}
                    dt = {}

                    def emit_s(P, jb):
                        """S matmuls (+ causal mask) for step (P, jb)."""
                        s_t = spool.tile([128, HPC, 256], FP32, tag="s")
                        diag0, diag1 = (jb == 2 * P), (jb == 2 * P + 1)
                        for h in range(HPC):
                            if diag1:
                                nc.tensor.matmul(
                                    s_t[:, h, 128:256],
                                    qkT[:, 1, h, jb * 128:(jb + 1) * 128],
                                    qkT[:, 0, h, P * 256 + 128:P * 256 + 256])
                            else:
                                nc.tensor.matmul(
                                    s_t[:, h, 0:256],
                                    qkT[:, 1, h, jb * 128:(jb + 1) * 128],
                                    qkT[:, 0, h, P * 256:(P + 1) * 256])
                        if diag0:
                            nc.vector.tensor_add(
                                s_t[:, :, 0:128], s_t[:, :, 0:128],
                                negmaskT.to_broadcast([128, HPC, 128]))
                        if diag1:
                            nc.vector.tensor_add(
                                s_t[:, :, 128:256], s_t[:, :, 128:256],
                                negmaskT.to_broadcast([128, HPC, 128]))
                        return s_t

                    s_cur = emit_s(*steps[0])
                    for si, (P, jb) in enumerate(steps):
                        if jb == 0:
                            ot[P] = opool.tile([128, HPC, 256], FP32, name="o_t", tag="o")
                            dt[P] = dpool.tile([128, HPC, 256], FP32, name="d_t", tag="d")
                        diag1 = (jb == 2 * P + 1)
                        s_next = (emit_s(*steps[si + 1])
                                  if si + 1 < len(steps) else None)
                        # exp split into h-pairs so den/PV of the first pair
                        # can start while the second half is still exp'ing
                        e_t = epool.tile([128, HPC, 256], BF16, tag="e")
                        cl, ch = (128, 256) if diag1 else (0, 256)
                        last = diag1
                        for hp in range(2):
                            h0 = 2 * hp
                            nc.scalar.activation(
                                out=e_t[:, h0:h0 + 2, cl:ch],
                                in_=s_cur[:, h0:h0 + 2, cl:ch],
                                func=mybir.ActivationFunctionType.Exp,
                                scale=QK_SCALE)
                            for h in (h0, h0 + 1):
                                nc.tensor.matmul(
                                    dt[P][:, h, cl:ch], ones_sb, e_t[:, h, cl:ch],
                                    start=(jb == 0 and h % 2 == 0), stop=last,
                                    skip_group_check=True)
                            for h in (h0, h0 + 1):
                                nc.tensor.matmul(
                                    ot[P][:, h, cl:ch],
                                    v_sb[:, jb, h * D:(h + 1) * D],
                                    e_t[:, h, cl:ch],
                                    start=(jb == 0 and h % 2 == 0), stop=last,
                                    skip_group_check=True)
                        if last:
                            # recip = 1/den on DVE (magic seed + 2 Newton
                            # steps) — keeps ACT free for the softmax exps
                            I32 = mybir.dt.int32
                            rec0 = rpool.tile([128, HPC, 256], FP32, tag="rec0")
                            nc.vector.tensor_scalar(
                                out=rec0.bitcast(I32), in0=dt[P].bitcast(I32),
                                scalar1=-1, scalar2=0x7EF311C3,
                                op0=mybir.AluOpType.mult, op1=mybir.AluOpType.add)
                            tn = rpool.tile([128, HPC, 256], FP32, tag="tn")
                            for _ in range(2):
                                nc.vector.tensor_mul(tn, dt[P], rec0)
                                nc.vector.tensor_scalar(
                                    out=tn, in0=tn, scalar1=-1.0, scalar2=2.0,
                                    op0=mybir.AluOpType.mult,
                                    op1=mybir.AluOpType.add)
                                nc.vector.tensor_mul(rec0, rec0, tn)
                            for hp in range(2):
                                h0 = 2 * hp
                                nc.vector.tensor_mul(
                                    oT[:, h0:h0 + 2, P * 256:(P + 1) * 256],
                                    ot[P][:, h0:h0 + 2, :], rec0[:, h0:h0 + 2, :])
                            del ot[P], dt[P]
                        s_cur = s_next

                # ---- phase 3: out = sum_h oT_h.T @ wo_h ----
                with ExitStack() as ph3:
                    upool = ph3.enter_context(
                        tc.tile_pool(name="u_ps", bufs=2, space=bass.MemorySpace.PSUM))
                    ostage = ph3.enter_context(tc.tile_pool(name="ostg", bufs=2))
                    for ti in range(NT):
                        ps_u = upool.tile([128, M], FP32, tag="u")
                        for mc in range(M // 512):
                            for h in range(HPC):
                                nc.tensor.matmul(
                                    ps_u[:, mc * 512:(mc + 1) * 512],
                                    oT[:, h, ti * 128:(ti + 1) * 128],
                                    wo_sb[:, h, mc * 512:(mc + 1) * 512],
                                    start=(h == 0), stop=(h == HPC - 1))
                        o_sb = ostage.tile([128, M], BF16, tag="osb")
                        nc.scalar.copy(out=o_sb[:, 0:1024], in_=ps_u[:, 0:1024])
                        nc.vector.tensor_copy(out=o_sb[:, 1024:2048],
                                              in_=ps_u[:, 1024:2048])
                        nc.sync.dma_start(out=out_d[ti * 128:(ti + 1) * 128, :],
                                          in_=o_sb)

        for _ in range(reps):
            _phase1()
            _phase23()

    return nc


def rope_consts(t_len=T):
    """cs_cs=[cos;sin], cs_sc=[sin;cos] ([128, t_len] bf16), c1=[I;-I], c2=[I;I]."""
    import ml_dtypes
    bf16 = ml_dtypes.bfloat16
    pos = np.arange(t_len, dtype=np.float64)[None, :]
    dims = np.arange(D // 2, dtype=np.float64)[:, None]
    freqs = ROTARY_BASE ** (-dims / (D // 2))
    rad = freqs * pos                              # [64, t_len]
    c, s = np.cos(rad), np.sin(rad)
    cs_cs = np.ascontiguousarray(np.concatenate([c, s]).astype(bf16))
    cs_sc = np.ascontiguousarray(np.concatenate([s, c]).astype(bf16))
    eye = np.eye(64, dtype=np.float32)
    c1 = np.ascontiguousarray(np.concatenate([eye, -eye]).astype(bf16))
    c2 = np.ascontiguousarray(np.concatenate([eye, eye]).astype(bf16))
    return cs_cs, cs_sc, c1, c2


_NC_CACHE = {}


def make_in_maps(x, wq, wk, wv, wo, t_len=T):
    import ml_dtypes
    bf16 = ml_dtypes.bfloat16
    fp8 = ml_dtypes.float8_e4m3
    cs_cs, cs_sc, c1, c2 = rope_consts(t_len)
    xTs = [np.ascontiguousarray(np.asarray(x[b]).T.astype(bf16)) for b in range(B)]
    # q/k projections are RMS-normed afterwards, so fp8 scaling cancels:
    # scale weights by 256 to clear the e4m3 subnormal range
    xq8s = [np.ascontiguousarray(np.asarray(x[b]).T.astype(fp8)) for b in range(B)]
    in_maps = []
    for c in range(N_CORES):
        b, g = divmod(c, N_CORES // B)
        hs = slice(g * HPC, (g + 1) * HPC)
        in_maps.append({
            "xT": xTs[b],
            "xq8": xq8s[b],
            "wq": np.ascontiguousarray(
                (wq[:, hs, :].reshape(M, JW) * 256.0).astype(fp8)),
            "wk": np.ascontiguousarray(
                (wk[:, hs, :].reshape(M, JW) * 256.0).astype(fp8)),
            "wv": np.ascontiguousarray(wv[:, hs, :].reshape(M, JW).astype(bf16)),
            "wo": np.ascontiguousarray(wo[hs].reshape(JW, M).astype(bf16)),
            "cs_cs": cs_cs, "cs_sc": cs_sc, "c1": c1, "c2": c2,
        })
    return in_maps


def kernel(x, wq, wk, wv, wo):
    if T not in _NC_CACHE:
        _NC_CACHE[T] = build_nc(T)
    nc = _NC_CACHE[T]
    in_maps = make_in_maps(x, wq, wk, wv, wo)
    res = run_bass_kernel_spmd(nc, in_maps, list(range(N_CORES)))
    gpb = N_CORES // B
    out = np.stack([
        sum(res.results[b * gpb + g]["out"].astype(np.float64) for g in range(gpb))
        for b in range(B)
    ]).astype(np.float32)
    return out
